# revision 1
# baseline (speedup 1.0000x reference)
"""Grouped Query Attention on 8 TRN2 NeuronCores.

Sharding: batch x s_q-quarter (core c -> batch c//4, query rows
[512*(c%4), 512*(c%4+1))). Each core computes the Q projection for its
512 query rows, the full KV projection for its batch (duplicated across
the 4 cores of that batch -- cheaper than collectives), attention for
all 16 heads over its query rows, and the output projection for a
disjoint [512, 2048] slice of the output. Unsharding is concatenation;
no collectives.

Layouts: all matmuls contract over the SBUF partition dim. Host
pre-transposes x to x^T (and rotates the core's own s_q quarter to the
front so one SPMD program serves all cores -- the s_k order inside
attention is permutation-invariant). Weights are pre-rearranged on host
so every DMA lands [128, ...] with >=2KB contiguous lines. Scores are
computed transposed ([s_k, s_q]) so softmax denominators come from an
M=1 ones-matmul and attn@V needs no transposes; 1/sqrt(128) is folded
into Wq on host. Matmul operands are bitcast to float32r (full PE
speed at free-dim >= 256, ~tf32 precision).
"""

import numpy as np

E = 2048
S = 2048
P = 128
H = 16
G = 4
SQ = 512          # query rows per core
EB = E // P       # 16 e-blocks (contraction tiles)
KV_N = 2 * E // G  # 1024
NCORES = 8

_NC = None
TRACE = False
LAST_RESULT = None


def _build():
    import concourse.bacc as bacc
    import concourse.mybir as mybir
    import concourse.tile as tile
    from concourse.masks import make_identity

    f32 = mybir.dt.float32
    f32r = mybir.dt.float32r
    bf16 = mybir.dt.bfloat16
    EXP = mybir.ActivationFunctionType.Exp

    nc = bacc.Bacc("TRN2", target_bir_lowering=False, debug=False,
                   num_devices=NCORES)

    xt = nc.declare_dram_parameter("xt", [P, EB, S], f32, isOutput=False).ap()
    wq = nc.declare_dram_parameter("wq", [H, P, EB, P], f32, isOutput=False).ap()
    wkv = nc.declare_dram_parameter("wkv", [P, EB, KV_N], f32, isOutput=False).ap()
    wo = nc.declare_dram_parameter("wo", [P, EB, E], f32, isOutput=False).ap()
    bq = nc.declare_dram_parameter("bq", [P, H], f32, isOutput=False).ap()
    bkv = nc.declare_dram_parameter("bkv", [P, 8], f32, isOutput=False).ap()
    bo = nc.declare_dram_parameter("bo", [1, E], f32, isOutput=False).ap()
    out = nc.declare_dram_parameter("out", [SQ, E], f32, isOutput=True).ap()

    def r(ap):
        return ap.bitcast(f32r)

    with tile.TileContext(nc) as tc:
        with tc.tile_pool(name="consts", bufs=1) as cp, \
             tc.tile_pool(name="otp", bufs=1) as otp, \
             tc.tile_pool(name="dram", bufs=1, space="DRAM") as dp:
            ident = cp.tile([P, P], f32, tag="ident")
            make_identity(nc, ident)
            onec = cp.tile([P, 1], bf16, tag="onec")
            nc.vector.memset(onec, 1.0)
            oner = cp.tile([1, P], f32, tag="oner")
            nc.vector.memset(oner, 1.0)
            bq_s = cp.tile([P, H], f32, tag="bqs")
            nc.sync.dma_start(bq_s, bq)
            bkv_s = cp.tile([P, 8], f32, tag="bkvs")
            nc.sync.dma_start(bkv_s, bkv)
            bo_s = cp.tile([1, E], f32, tag="bos")
            nc.sync.dma_start(bo_s, bo)

            OT = otp.tile([P, H, SQ], f32, tag="ot")   # normalized attn out, [hd, head, s_q]
            kvT = dp.tile([8 * P, S], f32, tag="kvt")  # K^T/V^T scratch, rows = kv M-tiles
            qtd = dp.tile([H * P, SQ], f32, tag="qtd") # Q^T scratch, rows = head blocks

            with tc.tile_pool(name="wkvp", bufs=1) as wkvp:
                wkv_s = wkvp.tile([P, EB, KV_N], f32, tag="wkvs")
                nc.sync.dma_start(r(wkv_s), r(wkv))

                # ---- Phase 1a: Q projection for this core's s_q quarter
                # (= chunk 0 of the rotated x^T). QT[do, s_q] accumulated
                # over 16 e-blocks, bias added on PSUM->SBUF, spilled to
                # DRAM scratch (re-streamed per head in phase 2).
                with tc.tile_pool(name="xqp", bufs=1) as xqp, \
                     tc.tile_pool(name="wqp", bufs=2) as wqp, \
                     tc.tile_pool(name="qop", bufs=2) as qop, \
                     tc.tile_pool(name="ps1", bufs=3, space="PSUM") as ps1:
                    xq = xqp.tile([P, EB, SQ], f32, tag="xq")
                    nc.sync.dma_start(r(xq), r(xt[:, :, 0:SQ]))
                    for m in range(H):
                        wqm = wqp.tile([P, EB, P], f32, tag="wqm")
                        nc.sync.dma_start(r(wqm), r(wq[m]))
                        ps = ps1.tile([P, SQ], f32, tag="ps")
                        for b in range(EB):
                            nc.tensor.matmul(ps, r(wqm[:, b]), r(xq[:, b]),
                                             start=(b == 0), stop=(b == EB - 1))
                        qo = qop.tile([P, SQ], f32, tag="qo")
                        nc.vector.tensor_scalar_add(qo, ps, bq_s[:, m:m + 1])
                        nc.sync.dma_start(qtd[m * P:(m + 1) * P, :], qo)

                # ---- Phase 1b: KV projection over the full sequence
                # (4 chunks of 512). M-tiles: [K0 V0 K1 V1 ...] matching
                # host Wkv column order.
                with tc.tile_pool(name="xcp", bufs=2) as xcp, \
                     tc.tile_pool(name="kvo", bufs=3) as kvo, \
                     tc.tile_pool(name="ps1b", bufs=3, space="PSUM") as ps1b:
                    for j in range(4):
                        xc = xcp.tile([P, EB, 512], f32, tag="xc")
                        nc.sync.dma_start(r(xc), r(xt[:, :, 512 * j:512 * (j + 1)]))
                        for m in range(8):
                            ps = ps1b.tile([P, 512], f32, tag="ps")
                            for b in range(EB):
                                nc.tensor.matmul(
                                    ps, r(wkv_s[:, b, m * P:(m + 1) * P]),
                                    r(xc[:, b]),
                                    start=(b == 0), stop=(b == EB - 1))
                            ko = kvo.tile([P, 512], f32, tag="ko")
                            nc.vector.tensor_scalar_add(ko, ps, bkv_s[:, m:m + 1])
                            nc.sync.dma_start(
                                kvT[m * P:(m + 1) * P, 512 * j:512 * (j + 1)], ko)

            # ---- Phase 2: attention, head group by head group.
            # scores^T[s_k, s_q] = K^T-tile.T @ Q^T -> exp on ACT ->
            # attn@V and ones-row-sum accumulate in PSUM over 16 s_k
            # tiles; normalize by broadcasting 1/l via a K=1 matmul.
            with tc.tile_pool(name="wop", bufs=2) as wop:
                won0 = wop.tile([P, EB, 512], f32, tag="won")
                nc.sync.dma_start(r(won0), r(wo[:, :, 0:512]))  # prefetch for phase 3

                with tc.tile_pool(name="kvl", bufs=2) as kvl, \
                     tc.tile_pool(name="vgp", bufs=2) as vgp, \
                     tc.tile_pool(name="qhp", bufs=3) as qhp, \
                     tc.tile_pool(name="exq", bufs=4) as exq, \
                     tc.tile_pool(name="lsb", bufs=2) as lsb, \
                     tc.tile_pool(name="pscp", bufs=3, space="PSUM") as pscp, \
                     tc.tile_pool(name="psop", bufs=2, space="PSUM") as psop, \
                     tc.tile_pool(name="pslp", bufs=1, space="PSUM") as pslp, \
                     tc.tile_pool(name="psbp", bufs=1, space="PSUM") as psbp:
                    for g in range(G):
                        kt = kvl.tile([P, S], f32, tag="kt")
                        nc.sync.dma_start(r(kt), r(kvT[2 * g * P:(2 * g + 1) * P, :]))
                        vt = kvl.tile([P, S], f32, tag="vt")
                        nc.sync.dma_start(vt, kvT[(2 * g + 1) * P:(2 * g + 2) * P, :])
                        vg = vgp.tile([P, 16, P], bf16, tag="vg")
                        for t in range(16):
                            pv = pscp.tile([P, P], f32, tag="psc")
                            nc.tensor.transpose(pv, vt[:, t * P:(t + 1) * P], ident)
                            nc.vector.tensor_copy(vg[:, t], pv)
                        for hl in range(4):
                            h = 4 * g + hl
                            qh = qhp.tile([P, SQ], f32, tag="qh")
                            nc.sync.dma_start(r(qh), r(qtd[h * P:(h + 1) * P, :]))
                            pso = psop.tile([P, SQ], f32, tag="pso")
                            psl = pslp.tile([1, SQ], f32, tag="psl")
                            exps = [None] * 16

                            def sc(t, qh=qh, kt=kt, exps=exps):
                                p = pscp.tile([P, SQ], f32, tag="psc")
                                nc.tensor.matmul(p, r(kt[:, t * P:(t + 1) * P]),
                                                 r(qh), start=True, stop=True)
                                e = exq.tile([P, SQ], bf16, tag="ex")
                                nc.scalar.activation(e, p, EXP)
                                exps[t] = e

                            sc(0)
                            sc(1)
                            for t in range(16):
                                if t + 2 < 16:
                                    sc(t + 2)
                                e = exps[t]
                                nc.tensor.matmul(pso, vg[:, t], e,
                                                 start=(t == 0), stop=(t == 15))
                                nc.tensor.matmul(psl, onec, e,
                                                 start=(t == 0), stop=(t == 15))
                            li = lsb.tile([1, SQ], f32, tag="li")
                            nc.vector.reciprocal(li, psl)
                            plb = psbp.tile([P, SQ], f32, tag="plb")
                            nc.tensor.matmul(plb, oner, li,
                                             start=True, stop=True)
                            lbs = lsb.tile([P, SQ], f32, tag="lbs")
                            nc.vector.tensor_copy(lbs, plb)
                            nc.vector.tensor_mul(r(OT[:, h]), pso, lbs)

                # ---- Phase 3: output projection. out[s_q, eo] accumulates
                # over 16 head blocks; bias seeded via a K=1 ones matmul.
                with tc.tile_pool(name="obp", bufs=3) as obp, \
                     tc.tile_pool(name="ps3", bufs=2, space="PSUM") as ps3p:
                    for n in range(4):
                        if n == 0:
                            won = won0
                        else:
                            won = wop.tile([P, EB, 512], f32, tag="won")
                            nc.sync.dma_start(r(won), r(wo[:, :, 512 * n:512 * (n + 1)]))
                        for ms in range(4):
                            ps = ps3p.tile([P, 512], f32, tag="ps")
                            nc.tensor.matmul(
                                ps, oner, bo_s[:, 512 * n:512 * (n + 1)],
                                start=True, stop=False)
                            for k in range(EB):
                                nc.tensor.matmul(
                                    ps, r(OT[:, k, ms * P:(ms + 1) * P]),
                                    r(won[:, k]),
                                    start=False, stop=(k == EB - 1))
                            ob = obp.tile([P, 512], f32, tag="ob")
                            nc.vector.tensor_copy(ob, ps)
                            nc.sync.dma_start(
                                out[ms * P:(ms + 1) * P, 512 * n:512 * (n + 1)], ob)

    nc.compile()
    return nc


def _get_nc():
    global _NC
    if _NC is None:
        _NC = _build()
    return _NC


def kernel(x, Wq, bq, Wkv, bkv, Wo, bo):
    from concourse.bass_utils import run_bass_kernel_spmd
    global LAST_RESULT

    x = np.asarray(x, np.float32)
    Wq = np.asarray(Wq, np.float32)
    bq = np.asarray(bq, np.float32)
    Wkv = np.asarray(Wkv, np.float32)
    bkv = np.asarray(bkv, np.float32)
    Wo = np.asarray(Wo, np.float32)
    bo = np.asarray(bo, np.float32)

    nc = _get_nc()
    sc = 1.0 / np.sqrt(E // H)
    # [m, p, b, d]: lhsT tile for Q M-tile m, e-block b
    wq_h = np.ascontiguousarray(
        (Wq * sc).reshape(EB, P, H, P).transpose(2, 1, 0, 3))
    wkv_h = np.ascontiguousarray(Wkv.reshape(EB, P, KV_N).transpose(1, 0, 2))
    wo_h = np.ascontiguousarray(Wo.reshape(EB, P, E).transpose(1, 0, 2))
    bq_h = np.ascontiguousarray((bq * sc).reshape(H, P).T)
    bkv_h = np.ascontiguousarray(bkv.reshape(8, P).T)
    bo_h = np.ascontiguousarray(bo.reshape(1, E))

    in_maps = []
    for c in range(NCORES):
        b, q = divmod(c, 4)
        xT = x[b].T  # [e, s]
        order = [q] + [i for i in range(4) if i != q]
        xtp = np.concatenate([xT[:, 512 * i:512 * (i + 1)] for i in order], axis=1)
        xt_h = np.ascontiguousarray(xtp.reshape(EB, P, S).transpose(1, 0, 2))
        in_maps.append({"xt": xt_h, "wq": wq_h, "wkv": wkv_h, "wo": wo_h,
                        "bq": bq_h, "bkv": bkv_h, "bo": bo_h})

    res = run_bass_kernel_spmd(nc, in_maps, core_ids=list(range(NCORES)),
                               trace=TRACE)
    LAST_RESULT = res

    outf = np.empty((2, S, E), np.float32)
    for c in range(NCORES):
        b, q = divmod(c, 4)
        outf[b, 512 * q:512 * (q + 1), :] = res.results[c]["out"]
    return outf



# revision 3
# speedup vs baseline: 1.3632x; 1.3632x over previous
"""Grouped Query Attention on 8 TRN2 NeuronCores.

Sharding: batch x s_q-quarter (core c -> batch c//4, query rows
[512*(c%4), 512*(c%4+1))). Each core computes the Q projection for its
512 query rows, the full KV projection for its batch (duplicated across
the 4 cores of that batch), attention for all 16 heads over its query
rows, and the output projection for a disjoint [512, 2048] slice of the
output. Unsharding is concatenation; no collectives.

v2 restructure vs baseline:
- All matmul inputs bf16 (halves DMA + SBUF; PE speed is the same as
  f32r). PSUM accumulation stays f32.
- Everything lives in SBUF between phases: x loaded once, Q^T / K^T /
  V(s-major) / attn outputs never round-trip through DRAM.
- V is projected directly in [s, d] orientation (lhsT = x^T s-tile) so
  phase 2 needs no PE transposes and no extra PSUM banks.
- Scores land in [P, 2, 512] PSUM tiles (2 banks) so each ACT exp
  instruction covers 1024 columns, amortizing fixed overheads.
- Per-head A-pass (scores+exp) / B-pass (attnV+denominator) software
  pipeline: A(h+1) is emitted before B(h) so the tensor engine never
  waits on the scalar engine and stays at full p-state clock.
- Normalization: ones-matmul denominator -> DVE reciprocal -> GPSIMD
  partition_broadcast -> DVE multiply (no fp32 broadcast matmuls).
- 1/sqrt(128) folded into Wq on host.
"""

import numpy as np

E = 2048
S = 2048
P = 128
H = 16
G = 4
SQ = 512          # query rows per core
EB = E // P       # 16 e-blocks (contraction tiles)
KV_N = 2 * E // G  # 1024
NCORES = 8

_NC = None
TRACE = False
LAST_RESULT = None


def _build():
    import concourse.bacc as bacc
    import concourse.mybir as mybir
    import concourse.tile as tile
    from concourse.masks import make_identity

    f32 = mybir.dt.float32
    bf16 = mybir.dt.bfloat16
    EXP = mybir.ActivationFunctionType.Exp
    IDENT = mybir.ActivationFunctionType.Identity

    nc = bacc.Bacc("TRN2", target_bir_lowering=False, debug=False,
                   num_devices=NCORES)

    # weights layouts (host pre-arranged):
    #   xt:  x^T rotated, [hd, eb, s]
    #   wq:  [head, p, eb, p]
    #   wkv: [p, eb, 1024] with columns [K0 K1 K2 K3 V0 V1 V2 V3]
    #   wo:  [p, eb, e]
    xt = nc.declare_dram_parameter("xt", [P, EB, S], bf16, isOutput=False).ap()
    wq = nc.declare_dram_parameter("wq", [H, P, EB, P], bf16, isOutput=False).ap()
    wkv = nc.declare_dram_parameter("wkv", [P, EB, KV_N], bf16, isOutput=False).ap()
    wo = nc.declare_dram_parameter("wo", [P, EB, E], bf16, isOutput=False).ap()
    bq = nc.declare_dram_parameter("bq", [P, H], f32, isOutput=False).ap()
    bkvk = nc.declare_dram_parameter("bkvk", [P, 4], f32, isOutput=False).ap()
    bkvv = nc.declare_dram_parameter("bkvv", [1, 512], bf16, isOutput=False).ap()
    bo = nc.declare_dram_parameter("bo", [1, E], bf16, isOutput=False).ap()
    out = nc.declare_dram_parameter("out", [SQ, E], f32, isOutput=True).ap()

    with tile.TileContext(nc) as tc:
        with tc.tile_pool(name="consts", bufs=1) as cp, \
             tc.tile_pool(name="qtsp", bufs=1) as qtsp, \
             tc.tile_pool(name="kvp", bufs=1) as kvp, \
             tc.tile_pool(name="otp", bufs=1) as otp:
            onec = cp.tile([P, 1], bf16, tag="onec")
            nc.vector.memset(onec, 1.0)
            oner = cp.tile([1, P], bf16, tag="oner")
            nc.vector.memset(oner, 1.0)
            bq_s = cp.tile([P, H], f32, tag="bqs")
            nc.sync.dma_start(bq_s, bq)
            bkvk_s = cp.tile([P, 4], f32, tag="bkvks")
            nc.sync.dma_start(bkvk_s, bkvk)
            bkvv_s = cp.tile([1, 512], bf16, tag="bkvvs")
            nc.sync.dma_start(bkvv_s, bkvv)
            bo_s = cp.tile([1, E], bf16, tag="bos")
            nc.sync.dma_start(bo_s, bo)

            qts = qtsp.tile([P, H, SQ], bf16, tag="qts")    # Q^T, [hd, head, sq]
            kts = kvp.tile([P, G, S], bf16, tag="kts")      # K^T, [hd, group, sk]
            vgs = kvp.tile([P, EB, 512], bf16, tag="vgs")   # V, [sk, sk_tile, g*128+hd]
            OT = otp.tile([P, H, SQ], bf16, tag="ot")       # attn out, [hd, head, sq]

            # ---- Phase 1: projections (all from SBUF-resident x^T).
            with tc.tile_pool(name="xsp", bufs=1) as xsp, \
                 tc.tile_pool(name="wkvp", bufs=1) as wkvp, \
                 tc.tile_pool(name="wqp", bufs=2) as wqp, \
                 tc.tile_pool(name="ps1", bufs=3, space="PSUM") as ps1, \
                 tc.tile_pool(name="ps1b", bufs=3, space="PSUM") as ps1b:
                xs = xsp.tile([P, EB, S], bf16, tag="xs")
                nc.sync.dma_start(xs[:, :, 0:SQ], xt[:, :, 0:SQ])
                nc.sync.dma_start(xs[:, :, SQ:S], xt[:, :, SQ:S])
                wkv_s = wkvp.tile([P, EB, KV_N], bf16, tag="wkvs")
                nc.sync.dma_start(wkv_s, wkv)

                # 1a: Q^T for this core's s_q quarter (chunk 0 of rotated x).
                for m in range(H):
                    wqm = wqp.tile([P, EB, P], bf16, tag="wqm")
                    nc.sync.dma_start(wqm, wq[m])
                    ps = ps1.tile([P, SQ], f32, tag="ps")
                    for b in range(EB):
                        nc.tensor.matmul(ps, wqm[:, b], xs[:, b, 0:SQ],
                                         start=(b == 0), stop=(b == EB - 1))
                    nc.vector.tensor_scalar_add(qts[:, m], ps, bq_s[:, m:m + 1])

                # 1b-K: K^T per group over the full (rotated) sequence.
                for m in range(G):
                    for j in range(4):
                        ps = ps1b.tile([P, 512], f32, tag="ps")
                        for b in range(EB):
                            nc.tensor.matmul(
                                ps, wkv_s[:, b, m * P:(m + 1) * P],
                                xs[:, b, 512 * j:512 * (j + 1)],
                                start=(b == 0), stop=(b == EB - 1))
                        # bias add on the scalar engine (idle in phase 1)
                        nc.scalar.activation(
                            kts[:, m, 512 * j:512 * (j + 1)], ps, IDENT,
                            bias=bkvk_s[:, m:m + 1])

                # 1b-V: V directly in [s, d] orientation: lhsT = x^T s-tile,
                # rhs = all V columns. Bias seeded via K=1 ones matmul.
                for t in range(EB):
                    ps = ps1b.tile([P, 512], f32, tag="ps")
                    nc.tensor.matmul(ps, oner, bkvv_s, start=True, stop=False)
                    for b in range(EB):
                        nc.tensor.matmul(
                            ps, xs[:, b, t * P:(t + 1) * P],
                            wkv_s[:, b, 512:KV_N],
                            start=False, stop=(b == EB - 1))
                    nc.vector.tensor_copy(vgs[:, t], ps)

            # ---- Phase 2: attention, A/B software pipeline over heads.
            with tc.tile_pool(name="wop", bufs=2) as wop, \
                 tc.tile_pool(name="eap", bufs=2) as eap, \
                 tc.tile_pool(name="lip", bufs=2) as lip, \
                 tc.tile_pool(name="lbp", bufs=2) as lbp:
                won0 = wop.tile([P, EB, 512], bf16, tag="won")
                nc.sync.dma_start(won0, wo[:, :, 0:512])  # prefetch for phase 3

                with tc.tile_pool(name="pscp", bufs=2, space="PSUM") as pscp, \
                     tc.tile_pool(name="psop", bufs=2, space="PSUM") as psop, \
                     tc.tile_pool(name="pslp", bufs=2, space="PSUM") as pslp:
                    eas = [None, None]

                    def a_pass(h):
                        g = h // 4
                        ea = eap.tile([P, EB, SQ], bf16, tag="ea")
                        for j in range(8):
                            ps2 = pscp.tile([P, 2, SQ], f32, tag="ps2")
                            for u in range(2):
                                t = 2 * j + u
                                nc.tensor.matmul(
                                    ps2[:, u], kts[:, g, t * P:(t + 1) * P],
                                    qts[:, h], start=True, stop=True)
                            nc.scalar.activation(ea[:, 2 * j:2 * j + 2], ps2, EXP)
                        eas[h % 2] = ea

                    def b_pass(h):
                        g = h // 4
                        ea = eas[h % 2]
                        pso = psop.tile([P, SQ], f32, tag="pso")
                        psl = pslp.tile([1, SQ], f32, tag="psl")
                        for t in range(EB):
                            nc.tensor.matmul(pso, vgs[:, t, g * P:(g + 1) * P],
                                             ea[:, t], start=(t == 0), stop=(t == EB - 1))
                            nc.tensor.matmul(psl, onec, ea[:, t],
                                             start=(t == 0), stop=(t == EB - 1))
                        li = lip.tile([1, SQ], f32, tag="li")
                        nc.vector.reciprocal(li, psl)
                        lb = lbp.tile([P, SQ], f32, tag="lb")
                        nc.gpsimd.partition_broadcast(lb, li)
                        nc.vector.tensor_mul(OT[:, h], pso, lb)

                    a_pass(0)
                    for h in range(H):
                        if h + 1 < H:
                            a_pass(h + 1)
                        b_pass(h)

                # ---- Phase 3: output projection, contraction over the 16
                # head blocks; bias seeded via a K=1 ones matmul.
                with tc.tile_pool(name="obp", bufs=3) as obp, \
                     tc.tile_pool(name="ps3", bufs=2, space="PSUM") as ps3p:
                    for n in range(4):
                        if n == 0:
                            won = won0
                        else:
                            won = wop.tile([P, EB, 512], bf16, tag="won")
                            nc.sync.dma_start(won, wo[:, :, 512 * n:512 * (n + 1)])
                        for ms in range(4):
                            ps = ps3p.tile([P, 512], f32, tag="ps")
                            nc.tensor.matmul(
                                ps, oner, bo_s[:, 512 * n:512 * (n + 1)],
                                start=True, stop=False)
                            for k in range(EB):
                                nc.tensor.matmul(
                                    ps, OT[:, k, ms * P:(ms + 1) * P],
                                    won[:, k],
                                    start=False, stop=(k == EB - 1))
                            ob = obp.tile([P, 512], f32, tag="ob")
                            nc.vector.tensor_copy(ob, ps)
                            nc.sync.dma_start(
                                out[ms * P:(ms + 1) * P, 512 * n:512 * (n + 1)], ob)

    nc.compile()
    return nc


def _get_nc():
    global _NC
    if _NC is None:
        _NC = _build()
    return _NC


def kernel(x, Wq, bq, Wkv, bkv, Wo, bo):
    from concourse.bass_utils import run_bass_kernel_spmd
    import ml_dtypes
    global LAST_RESULT

    bf = ml_dtypes.bfloat16
    x = np.asarray(x, np.float32)
    Wq = np.asarray(Wq, np.float32)
    bq = np.asarray(bq, np.float32)
    Wkv = np.asarray(Wkv, np.float32)
    bkv = np.asarray(bkv, np.float32)
    Wo = np.asarray(Wo, np.float32)
    bo = np.asarray(bo, np.float32)

    nc = _get_nc()
    sc = 1.0 / np.sqrt(E // H)
    # wq: [m, p, b, d] lhsT tiles, scale folded
    wq_h = np.ascontiguousarray(
        (Wq * sc).reshape(EB, P, H, P).transpose(2, 1, 0, 3)).astype(bf)
    # wkv: columns regrouped to [K0 K1 K2 K3 | V0 V1 V2 V3]
    kcols = np.concatenate([Wkv[:, 256 * g:256 * g + 128] for g in range(G)], axis=1)
    vcols = np.concatenate([Wkv[:, 256 * g + 128:256 * g + 256] for g in range(G)], axis=1)
    wkv_re = np.concatenate([kcols, vcols], axis=1)  # [E, 1024]
    wkv_h = np.ascontiguousarray(wkv_re.reshape(EB, P, KV_N).transpose(1, 0, 2)).astype(bf)
    wo_h = np.ascontiguousarray(Wo.reshape(EB, P, E).transpose(1, 0, 2)).astype(bf)
    bq_h = np.ascontiguousarray((bq * sc).reshape(H, P).T).astype(np.float32)
    bkv_k = np.stack([bkv[256 * g:256 * g + 128] for g in range(G)], axis=1)  # [128, 4]
    bkv_v = np.concatenate([bkv[256 * g + 128:256 * g + 256] for g in range(G)])  # [512]
    bkvk_h = np.ascontiguousarray(bkv_k).astype(np.float32)
    bkvv_h = np.ascontiguousarray(bkv_v.reshape(1, 512)).astype(bf)
    bo_h = np.ascontiguousarray(bo.reshape(1, E)).astype(bf)

    in_maps = []
    for c in range(NCORES):
        b, q = divmod(c, 4)
        xT = x[b].T  # [e, s]
        order = [q] + [i for i in range(4) if i != q]
        xtp = np.concatenate([xT[:, 512 * i:512 * (i + 1)] for i in order], axis=1)
        xt_h = np.ascontiguousarray(
            xtp.reshape(EB, P, S).transpose(1, 0, 2)).astype(bf)
        in_maps.append({"xt": xt_h, "wq": wq_h, "wkv": wkv_h, "wo": wo_h,
                        "bq": bq_h, "bkvk": bkvk_h, "bkvv": bkvv_h, "bo": bo_h})

    res = run_bass_kernel_spmd(nc, in_maps, core_ids=list(range(NCORES)),
                               trace=TRACE)
    LAST_RESULT = res

    outf = np.empty((2, S, E), np.float32)
    for c in range(NCORES):
        b, q = divmod(c, 4)
        outf[b, 512 * q:512 * (q + 1), :] = res.results[c]["out"]
    return outf


# revision 5
# speedup vs baseline: 1.4324x; 1.0507x over previous
"""Grouped Query Attention on 8 TRN2 NeuronCores.

Sharding: batch x s_q-quarter (core c -> batch c//4, query rows
[512*(c%4), 512*(c%4+1))). Each core computes the Q projection for its
512 query rows, attention for all 16 heads over its query rows, and the
output projection for a disjoint [512, 2048] slice of the output.

The KV projection is sharded: each core projects K^T and V only for its
OWN sequence quarter (= chunk 0 of its rotated x), packs them into a
1 MB DRAM buffer, and a 4-core AllGather per batch assembles the full
K^T/V in canonical sequence order while the tensor engine runs the Q
projection. Attention consumes the gathered K/V (s_k order is
permutation-invariant; K and V share the canonical order).

Other structure (v2):
- All matmul inputs bf16; PSUM accumulation f32.
- x chunk / Q^T / K^T / V / attn outputs are SBUF-resident.
- V is projected directly in [s, d] orientation (lhsT = x^T s-tile) so
  phase 2 needs no PE transposes.
- Scores land in [P, 2, 512] PSUM tiles so each ACT exp instruction
  covers 1024 columns.
- Per-head A-pass (scores+exp) / B-pass (attnV+denominator) software
  pipeline keeps the tensor engine dense (full p-state clock).
- Normalization: ones-matmul denominator -> DVE reciprocal -> GPSIMD
  partition_broadcast -> DVE multiply.
- Bulk weight loads ride the ACT-engine DMA queue so the SP queue only
  carries the latency-critical stream (x chunk, per-head Q weights).
- 1/sqrt(128) folded into Wq on host.
"""

import numpy as np

E = 2048
S = 2048
P = 128
H = 16
G = 4
SQ = 512          # query rows per core
EB = E // P       # 16 e-blocks (contraction tiles)
KV_N = 2 * E // G  # 1024
NCORES = 8

_NC = None
TRACE = False
LAST_RESULT = None


def _build():
    import concourse.bacc as bacc
    import concourse.mybir as mybir
    import concourse.tile as tile

    f32 = mybir.dt.float32
    bf16 = mybir.dt.bfloat16
    EXP = mybir.ActivationFunctionType.Exp
    IDENT = mybir.ActivationFunctionType.Identity

    nc = bacc.Bacc("TRN2", target_bir_lowering=False, debug=False,
                   num_devices=NCORES)

    # host layouts:
    #   xt:  x^T rotated chunk 0 (this core's quarter), [hd, eb, s_own]
    #   wq:  [head, p, eb, p] (1/sqrt(d) folded)
    #   wkv: [p, eb, 1024] with columns [K0 K1 K2 K3 V0 V1 V2 V3]
    #   wo:  [p, eb, e]
    xt = nc.declare_dram_parameter("xt", [P, EB, SQ], bf16, isOutput=False).ap()
    wq = nc.declare_dram_parameter("wq", [H, P, EB, P], bf16, isOutput=False).ap()
    wkv = nc.declare_dram_parameter("wkv", [P, EB, KV_N], bf16, isOutput=False).ap()
    wo = nc.declare_dram_parameter("wo", [P, EB, E], bf16, isOutput=False).ap()
    bq = nc.declare_dram_parameter("bq", [P, H], f32, isOutput=False).ap()
    bkvk = nc.declare_dram_parameter("bkvk", [P, 4], f32, isOutput=False).ap()
    bkvv = nc.declare_dram_parameter("bkvv", [1, 512], bf16, isOutput=False).ap()
    bo = nc.declare_dram_parameter("bo", [1, E], bf16, isOutput=False).ap()
    out = nc.declare_dram_parameter("out", [SQ, E], f32, isOutput=True).ap()

    RG = [[0, 1, 2, 3], [4, 5, 6, 7]]

    with tile.TileContext(nc) as tc:
        with tc.tile_pool(name="consts", bufs=1) as cp, \
             tc.tile_pool(name="qtsp", bufs=1) as qtsp, \
             tc.tile_pool(name="kvp", bufs=1) as kvp, \
             tc.tile_pool(name="otp", bufs=1) as otp, \
             tc.tile_pool(name="dram", bufs=1, space="DRAM") as dp:
            onec = cp.tile([P, 1], bf16, tag="onec")
            nc.vector.memset(onec, 1.0)
            oner = cp.tile([1, P], bf16, tag="oner")
            nc.vector.memset(oner, 1.0)
            bq_s = cp.tile([P, H], f32, tag="bqs")
            nc.sync.dma_start(bq_s, bq)
            bkvk_s = cp.tile([P, 4], f32, tag="bkvks")
            nc.sync.dma_start(bkvk_s, bkvk)
            bkvv_s = cp.tile([1, 512], bf16, tag="bkvvs")
            nc.sync.dma_start(bkvv_s, bkvv)
            bo_s = cp.tile([1, E], bf16, tag="bos")
            nc.sync.dma_start(bo_s, bo)

            qts = qtsp.tile([P, H, SQ], bf16, tag="qts")    # Q^T, [hd, head, sq]
            kts = kvp.tile([P, G, S], bf16, tag="kts")      # K^T, [hd, group, sk]
            vgs = kvp.tile([P, EB, 512], bf16, tag="vgs")   # V, [sk, sk_tile, g*128+hd]
            OT = otp.tile([P, H, SQ], bf16, tag="ot")       # attn out, [hd, head, sq]

            # own-quarter KV pack: m 0..3 = K^T groups, m 4..7 = V s-tiles
            kvown = dp.tile([P, 8, 512], bf16, tag="kvown")
            kvall = dp.tile([4, P, 8, 512], bf16, tag="kvall")

            # ---- Phase 1: projections from the SBUF-resident x^T quarter.
            with tc.tile_pool(name="xsp", bufs=1) as xsp, \
                 tc.tile_pool(name="wkvp", bufs=1) as wkvp, \
                 tc.tile_pool(name="kvsg", bufs=1) as kvsg, \
                 tc.tile_pool(name="wqp", bufs=2) as wqp, \
                 tc.tile_pool(name="ps1", bufs=3, space="PSUM") as ps1, \
                 tc.tile_pool(name="ps1b", bufs=3, space="PSUM") as ps1b:
                xs = xsp.tile([P, EB, SQ], bf16, tag="xs")
                nc.sync.dma_start(xs, xt)
                wkv_s = wkvp.tile([P, EB, KV_N], bf16, tag="wkvs")
                nc.scalar.dma_start(wkv_s, wkv)  # bulk load on ACT queue
                kvstg = kvsg.tile([P, 8, 512], bf16, tag="kvstg")

                def q_head(m):
                    wqm = wqp.tile([P, EB, P], bf16, tag="wqm")
                    nc.sync.dma_start(wqm, wq[m])
                    ps = ps1.tile([P, SQ], f32, tag="ps")
                    for b in range(EB):
                        nc.tensor.matmul(ps, wqm[:, b], xs[:, b],
                                         start=(b == 0), stop=(b == EB - 1))
                    nc.vector.tensor_scalar_add(qts[:, m], ps, bq_s[:, m:m + 1])

                # two Q heads first so the PE starts as soon as possible
                q_head(0)
                q_head(1)

                # K^T for all 4 groups over this core's own quarter
                for m in range(G):
                    ps = ps1b.tile([P, 512], f32, tag="ps")
                    for b in range(EB):
                        nc.tensor.matmul(
                            ps, wkv_s[:, b, m * P:(m + 1) * P], xs[:, b],
                            start=(b == 0), stop=(b == EB - 1))
                    nc.scalar.activation(kvstg[:, m], ps, IDENT,
                                         bias=bkvk_s[:, m:m + 1])

                # V in [s, d] orientation for this core's own 4 s-tiles
                for t in range(4):
                    ps = ps1b.tile([P, 512], f32, tag="ps")
                    nc.tensor.matmul(ps, oner, bkvv_s, start=True, stop=False)
                    for b in range(EB):
                        nc.tensor.matmul(
                            ps, xs[:, b, t * P:(t + 1) * P],
                            wkv_s[:, b, 512:KV_N],
                            start=False, stop=(b == EB - 1))
                    nc.vector.tensor_copy(kvstg[:, 4 + t], ps)

                # pack -> DRAM -> AllGather (runs while Q projection continues)
                nc.sync.dma_start(kvown, kvstg)
                nc.gpsimd.collective_compute(
                    "AllGather", mybir.AluOpType.bypass,
                    replica_groups=RG, ins=[kvown[:]], outs=[kvall[:]])
                for g in range(G):
                    nc.sync.dma_start(
                        kts[:, g], kvall[:, :, g].rearrange("c p w -> p c w"))
                nc.sync.dma_start(
                    vgs, kvall[:, :, 4:8].rearrange("c p i w -> p c i w"))

                # remaining Q heads overlap the collective
                for m in range(2, H):
                    q_head(m)

            # ---- Phase 2: attention, A/B software pipeline over heads.
            with tc.tile_pool(name="wop", bufs=2) as wop, \
                 tc.tile_pool(name="eap", bufs=2) as eap, \
                 tc.tile_pool(name="lip", bufs=2) as lip, \
                 tc.tile_pool(name="lbp", bufs=2) as lbp:
                won0 = wop.tile([P, EB, 512], bf16, tag="won")
                nc.scalar.dma_start(won0, wo[:, :, 0:512])  # prefetch phase 3

                with tc.tile_pool(name="pscp", bufs=2, space="PSUM") as pscp, \
                     tc.tile_pool(name="psop", bufs=2, space="PSUM") as psop, \
                     tc.tile_pool(name="pslp", bufs=2, space="PSUM") as pslp:
                    eas = [None, None]

                    def a_pass(h):
                        g = h // 4
                        ea = eap.tile([P, EB, SQ], bf16, tag="ea")
                        for j in range(8):
                            ps2 = pscp.tile([P, 2, SQ], f32, tag="ps2")
                            for u in range(2):
                                t = 2 * j + u
                                nc.tensor.matmul(
                                    ps2[:, u], kts[:, g, t * P:(t + 1) * P],
                                    qts[:, h], start=True, stop=True)
                            nc.scalar.activation(ea[:, 2 * j:2 * j + 2], ps2, EXP)
                        eas[h % 2] = ea

                    def b_pass(h):
                        g = h // 4
                        ea = eas[h % 2]
                        pso = psop.tile([P, SQ], f32, tag="pso")
                        psl = pslp.tile([1, SQ], f32, tag="psl")
                        for t in range(EB):
                            nc.tensor.matmul(pso, vgs[:, t, g * P:(g + 1) * P],
                                             ea[:, t], start=(t == 0), stop=(t == EB - 1))
                            nc.tensor.matmul(psl, onec, ea[:, t],
                                             start=(t == 0), stop=(t == EB - 1))
                        li = lip.tile([1, SQ], f32, tag="li")
                        nc.vector.reciprocal(li, psl)
                        lb = lbp.tile([P, SQ], f32, tag="lb")
                        nc.gpsimd.partition_broadcast(lb, li)
                        nc.vector.tensor_mul(OT[:, h], pso, lb)

                    a_pass(0)
                    for h in range(H):
                        if h + 1 < H:
                            a_pass(h + 1)
                        b_pass(h)

                # ---- Phase 3: output projection, contraction over the 16
                # head blocks; bias seeded via a K=1 ones matmul.
                with tc.tile_pool(name="obp", bufs=3) as obp, \
                     tc.tile_pool(name="ps3", bufs=2, space="PSUM") as ps3p:
                    for n in range(4):
                        if n == 0:
                            won = won0
                        else:
                            won = wop.tile([P, EB, 512], bf16, tag="won")
                            nc.scalar.dma_start(won, wo[:, :, 512 * n:512 * (n + 1)])
                        for ms in range(4):
                            ps = ps3p.tile([P, 512], f32, tag="ps")
                            nc.tensor.matmul(
                                ps, oner, bo_s[:, 512 * n:512 * (n + 1)],
                                start=True, stop=False)
                            for k in range(EB):
                                nc.tensor.matmul(
                                    ps, OT[:, k, ms * P:(ms + 1) * P],
                                    won[:, k],
                                    start=False, stop=(k == EB - 1))
                            ob = obp.tile([P, 512], f32, tag="ob")
                            nc.vector.tensor_copy(ob, ps)
                            nc.sync.dma_start(
                                out[ms * P:(ms + 1) * P, 512 * n:512 * (n + 1)], ob)

    nc.compile()
    return nc


def _get_nc():
    global _NC
    if _NC is None:
        _NC = _build()
    return _NC


def kernel(x, Wq, bq, Wkv, bkv, Wo, bo):
    from concourse.bass_utils import run_bass_kernel_spmd
    import ml_dtypes
    global LAST_RESULT

    bf = ml_dtypes.bfloat16
    x = np.asarray(x, np.float32)
    Wq = np.asarray(Wq, np.float32)
    bq = np.asarray(bq, np.float32)
    Wkv = np.asarray(Wkv, np.float32)
    bkv = np.asarray(bkv, np.float32)
    Wo = np.asarray(Wo, np.float32)
    bo = np.asarray(bo, np.float32)

    nc = _get_nc()
    sc = 1.0 / np.sqrt(E // H)
    wq_h = np.ascontiguousarray(
        (Wq * sc).reshape(EB, P, H, P).transpose(2, 1, 0, 3)).astype(bf)
    kcols = np.concatenate([Wkv[:, 256 * g:256 * g + 128] for g in range(G)], axis=1)
    vcols = np.concatenate([Wkv[:, 256 * g + 128:256 * g + 256] for g in range(G)], axis=1)
    wkv_re = np.concatenate([kcols, vcols], axis=1)  # [E, 1024]
    wkv_h = np.ascontiguousarray(wkv_re.reshape(EB, P, KV_N).transpose(1, 0, 2)).astype(bf)
    wo_h = np.ascontiguousarray(Wo.reshape(EB, P, E).transpose(1, 0, 2)).astype(bf)
    bq_h = np.ascontiguousarray((bq * sc).reshape(H, P).T).astype(np.float32)
    bkv_k = np.stack([bkv[256 * g:256 * g + 128] for g in range(G)], axis=1)
    bkv_v = np.concatenate([bkv[256 * g + 128:256 * g + 256] for g in range(G)])
    bkvk_h = np.ascontiguousarray(bkv_k).astype(np.float32)
    bkvv_h = np.ascontiguousarray(bkv_v.reshape(1, 512)).astype(bf)
    bo_h = np.ascontiguousarray(bo.reshape(1, E)).astype(bf)

    in_maps = []
    for c in range(NCORES):
        b, q = divmod(c, 4)
        xq = x[b, 512 * q:512 * (q + 1), :].T  # [e, s_own] — own quarter only
        xt_h = np.ascontiguousarray(
            xq.reshape(EB, P, SQ).transpose(1, 0, 2)).astype(bf)
        in_maps.append({"xt": xt_h, "wq": wq_h, "wkv": wkv_h, "wo": wo_h,
                        "bq": bq_h, "bkvk": bkvk_h, "bkvv": bkvv_h, "bo": bo_h})

    res = run_bass_kernel_spmd(nc, in_maps, core_ids=list(range(NCORES)),
                               trace=TRACE)
    LAST_RESULT = res

    outf = np.empty((2, S, E), np.float32)
    for c in range(NCORES):
        b, q = divmod(c, 4)
        outf[b, 512 * q:512 * (q + 1), :] = res.results[c]["out"]
    return outf
